# revision 9
# baseline (speedup 1.0000x reference)
"""Trainium2 Bass kernel for the 2-layer LSTM greedy decoder (nn_Decoder).

Strategy: data-parallel over batch (4096 -> 512 per core x 8 cores).
All recurrent state is kept feature-major in SBUF ([H partitions, batch
free]). Gate matmuls run in fp8-e4m3 with DoubleRow perf mode (2 fp8
weights per PE cell, 2x ALU rate): weights are host-scaled by 1024 into
e4m3 range and the 1/1024 descale rides the scalar-engine activation's
`scale` operand; h state is stored e4m3 (|h|<1 is native range). All LSTM
weights (12MB fp8) stay resident in SBUF - no steady-state HBM traffic.
The logits path keeps full precision: a bf16 copy of h1 feeds the fc
matmul. Argmax feedback is folded into the layer-0 gate accumulation as a
rank-2 f32r matmul update (ones/m rows, pre-scaled by 1024); biases ride
the activation's per-partition bias operand (applied after descale).
"""

import sys

sys.path.insert(0, "/opt/trn_rl_repo")

import numpy as np
import ml_dtypes

import concourse.bass as bass
import concourse.bacc as bacc
import concourse.mybir as mybir
import concourse.tile as tile
from concourse.bass_utils import run_bass_kernel_spmd

F32 = mybir.dt.float32
F32R = mybir.dt.float32r
BF16 = mybir.dt.bfloat16
FP8 = mybir.dt.float8e4
AF = mybir.ActivationFunctionType
ALU = mybir.AluOpType
DR = mybir.MatmulPerfMode.DoubleRow

H = 1024
B = 4096
C = 2
NCORES = 8
BS = B // NCORES          # 512 batch per core
KT = H // 128             # 8 k-tiles
HB = H // 128             # 8 hidden blocks
NQ = 4                    # i, f, g, o

WS = 1024.0               # host-side weight scale into e4m3 range
WSI = 1.0 / WS

BF = ml_dtypes.bfloat16
E4 = ml_dtypes.float8_e4m3


def _round_f32r(x: np.ndarray) -> np.ndarray:
    """Round fp32 to the PE's FP22 (13-bit mantissa) operand precision."""
    u = np.ascontiguousarray(x, dtype=np.float32).view(np.uint32)
    u = (u + np.uint32(0x200)) & np.uint32(0xFFFFFC00)
    return u.view(np.float32)


def build_kernel(T: int, unroll_all: bool = False):
    nc = bacc.Bacc(None, target_bir_lowering=False)

    hs = nc.dram_tensor("hs", [2, 128, KT, BS], FP8, kind="ExternalInput")
    hs1b = nc.dram_tensor("hs1b", [128, KT, BS], BF16, kind="ExternalInput")
    hs0b = nc.dram_tensor("hs0b", [128, KT, BS], BF16, kind="ExternalInput")
    w0pb = nc.dram_tensor("w0pb", [HB, NQ, 128, KT, 128], BF16, kind="ExternalInput")
    w1pb = nc.dram_tensor("w1pb", [HB, NQ, 128, 2 * KT, 128], BF16,
                          kind="ExternalInput")
    cs = nc.dram_tensor("cs", [2, 128, KT, BS], BF16, kind="ExternalInput")
    w0p = nc.dram_tensor("w0p", [HB, NQ, 128, KT, 128], FP8, kind="ExternalInput")
    w1p = nc.dram_tensor("w1p", [HB, NQ, 128, 2 * KT, 128], FP8, kind="ExternalInput")
    xf = nc.dram_tensor("xf", [128, HB * NQ * 128], FP8, kind="ExternalInput")
    b0 = nc.dram_tensor("b0", [128, HB * NQ], F32, kind="ExternalInput")
    b1 = nc.dram_tensor("b1", [128, HB * NQ], F32, kind="ExternalInput")
    fcw = nc.dram_tensor("fcw", [128, KT, 3], BF16, kind="ExternalInput")
    fcb = nc.dram_tensor("fcb", [1, 3], BF16, kind="ExternalInput")
    ident = nc.dram_tensor("ident", [128, 128], F32, kind="ExternalInput")
    onesb = nc.dram_tensor("onesb", [1, BS], BF16, kind="ExternalInput")
    mu0 = nc.dram_tensor("mu0", [128, BS], FP8, kind="ExternalInput")
    lout = nc.dram_tensor("lout", [4, 128, T, 2], F32, kind="ExternalOutput")

    with tile.TileContext(nc) as tc:
        with (
            tc.tile_pool(name="st", bufs=1) as st,
            tc.tile_pool(name="wst", bufs=2) as wst,
            tc.tile_pool(name="tmp", bufs=3) as tmp,
            tc.tile_pool(name="gps", bufs=6, space="PSUM") as gps,
            tc.tile_pool(name="lps", bufs=1, space="PSUM") as lpsp,
            tc.tile_pool(name="tps", bufs=1, space="PSUM") as tpsp,
        ):
            # Persistent state (ping-pong h buffers; c updated in place)
            h0a = st.tile([128, KT, BS], FP8, tag="h0a")
            h0b = st.tile([128, KT, BS], FP8, tag="h0b")
            h1a = st.tile([128, KT, BS], FP8, tag="h1a")
            h1b = st.tile([128, KT, BS], FP8, tag="h1b")
            h0ba = st.tile([128, KT, BS], BF16, tag="h0ba")  # bf16 h0 init
            h0nb = st.tile([128, KT, BS], BF16, tag="h0nb")  # bf16 h0n (step 0)
            h1ba = st.tile([128, KT, BS], BF16, tag="h1ba")  # bf16 h1 (logits)
            h1bb = st.tile([128, KT, BS], BF16, tag="h1bb")
            c0 = st.tile([128, KT, BS], BF16, tag="c0")
            c1 = st.tile([128, KT, BS], BF16, tag="c1")
            w0_sb = st.tile([128, HB, NQ, KT, 128], FP8, tag="w0")
            w1_sb = st.tile([128, HB, NQ, 2 * KT, 128], FP8, tag="w1")
            xf_sb = st.tile([128, HB * NQ * 128], FP8, tag="xf")
            b0_sb = st.tile([128, HB * NQ], F32, tag="b0")
            b1_sb = st.tile([128, HB * NQ], F32, tag="b1")
            fcw_sb = st.tile([128, KT, 3], BF16, tag="fcw")
            fcb_sb = st.tile([1, 3], BF16, tag="fcb")
            id_sb = st.tile([128, 128], F32, tag="ident")
            mu_sb = st.tile([128, BS], FP8, tag="mu")  # rows 32q=m, 32q+1=ones
            ones_sb = st.tile([1, BS], BF16, tag="ones")
            lt_sb = st.tile([3, BS], F32, tag="lt")    # rows d, l0, l1
            lacc = st.tile([128, 4 * T * 2], F32, tag="lacc")

            nc.sync.dma_start(h0a[:], hs[0])
            nc.sync.dma_start(h1a[:], hs[1])
            nc.sync.dma_start(h1ba[:], hs1b[:])
            nc.sync.dma_start(h0ba[:], hs0b[:])
            nc.sync.dma_start(c0[:], cs[0])
            nc.sync.dma_start(c1[:], cs[1])
            for j in range(HB):
                for q in range(NQ):
                    nc.sync.dma_start(w0_sb[:, j, q], w0p[j, q])
                    nc.sync.dma_start(w1_sb[:, j, q], w1p[j, q])
            nc.sync.dma_start(xf_sb[:], xf[:])
            nc.sync.dma_start(b0_sb[:], b0[:])
            nc.sync.dma_start(b1_sb[:], b1[:])
            nc.sync.dma_start(fcw_sb[:], fcw[:])
            nc.sync.dma_start(fcb_sb[:], fcb[:])
            nc.sync.dma_start(id_sb[:], ident[:])
            # row1 stays 1.0 forever; row0 (m) is overwritten by is_lt each
            # step before any matmul reads it (step 0 skips the m-matmul).
            nc.sync.dma_start(mu_sb[:], mu0[:])
            nc.sync.dma_start(ones_sb[:], onesb[:])

            def phase_gates(layer, j, h_in, h_aux, with_m):
                """Gate matmuls + activations for hidden block j of one layer.

                layer 0: contraction = W_hh0 @ h_in (+ x feedback via m-matmul)
                layer 1: contraction = W_hh1 @ h_in then W_ih1 @ h_aux
                All gate matmuls are fp8 DoubleRow over k-tile pairs.
                """
                nkt = KT if layer == 0 else 2 * KT
                wsrc = w0_sb if layer == 0 else w1_sb
                bias = b0_sb if layer == 0 else b1_sb
                gact = tmp.tile([128, NQ, BS], BF16, tag="gact")
                for q in range(NQ):
                    w = wsrc[:, j, q]
                    ps = gps.tile([128, BS], F32, tag="g")
                    for m in range(nkt // 2):
                        rhs = h_in if m < KT // 2 else h_aux
                        kk = (2 * m) % KT
                        last = (m == nkt // 2 - 1) and not (layer == 0 and with_m)
                        nc.tensor.matmul(
                            ps[:],
                            w[:, 2 * m:2 * m + 2, :],
                            rhs[:, kk:kk + 2, :],
                            start=(m == 0),
                            stop=last,
                            perf_mode=DR,
                        )
                    if layer == 0 and with_m:
                        idx = j * NQ + q
                        nc.tensor.matmul(
                            ps[:],
                            xf_sb[32 * q:32 * q + 2, idx * 128:(idx + 1) * 128],
                            mu_sb[32 * q:32 * q + 2, :],
                            start=False,
                            stop=True,
                            tile_position=(32 * q, 0),
                        )
                    fn = AF.Tanh if q == 2 else AF.Sigmoid
                    idx = j * NQ + q
                    nc.scalar.activation(
                        gact[:, q, :], ps[:], fn,
                        bias=bias[:, idx:idx + 1], scale=WSI,
                    )
                return gact

            def phase_gates_precise(layer, j, h_in, h_aux):
                """Step-0 gate matmuls in bf16 (initial h is N(0,1)-scale;
                fp8 would inject a large decaying transient)."""
                nkt = KT if layer == 0 else 2 * KT
                wsrc = w0pb if layer == 0 else w1pb
                bias = b0_sb if layer == 0 else b1_sb
                gact = tmp.tile([128, NQ, BS], BF16, tag="gact")
                for q in range(NQ):
                    w = wst.tile([128, 2 * KT, 128], BF16, tag="w")
                    nc.sync.dma_start(w[:, :nkt, :], wsrc[j, q])
                    ps = gps.tile([128, BS], F32, tag="g")
                    for kt in range(nkt):
                        rhs = h_in if kt < KT else h_aux
                        nc.tensor.matmul(
                            ps[:],
                            w[:, kt, :],
                            rhs[:, kt % KT, :],
                            start=(kt == 0),
                            stop=(kt == nkt - 1),
                        )
                    fn = AF.Tanh if q == 2 else AF.Sigmoid
                    idx = j * NQ + q
                    nc.scalar.activation(
                        gact[:, q, :], ps[:], fn, bias=bias[:, idx:idx + 1]
                    )
                return gact

            def phase_cell(j, gact, c_st, h_out, h_out_b=None):
                t1 = tmp.tile([128, BS], BF16, tag="t1")
                t2 = tmp.tile([128, BS], BF16, tag="t2")
                ct = tmp.tile([128, BS], BF16, tag="ct")
                nc.vector.tensor_mul(t1[:], gact[:, 1, :], c_st[:, j, :])
                nc.vector.tensor_mul(t2[:], gact[:, 0, :], gact[:, 2, :])
                nc.vector.tensor_add(c_st[:, j, :], t1[:], t2[:])
                nc.scalar.activation(ct[:], c_st[:, j, :], AF.Tanh)
                nc.vector.tensor_mul(h_out[:, j, :], gact[:, 3, :], ct[:])
                if h_out_b is not None:
                    nc.vector.tensor_mul(h_out_b[:, j, :], gact[:, 3, :], ct[:])

            def emit_step(t_off, h0_in, h0_out, h1_in, h1_out, h1b_in, h1b_out,
                          first):
                gates0 = (phase_gates_precise if first else
                          lambda l, j, hi, ha: phase_gates(l, j, hi, ha, False))
                # one-block skew: block j's cell tail is emitted after block
                # j+1's gate phase, keeping tanh(c)'s DVE-wait off the ACT
                # engine's head-of-line and freeing PSUM banks promptly.
                h0_out_b = h0nb if first else None
                prev = None
                for j in range(HB):
                    if first:
                        g = gates0(0, j, h0_in, None)
                    else:
                        g = phase_gates(0, j, h0_in, None, True)
                    if prev is not None:
                        phase_cell(prev[0], prev[1], c0, h0_out, h0_out_b)
                    prev = (j, g)
                phase_cell(prev[0], prev[1], c0, h0_out, h0_out_b)
                prev = None
                for j in range(HB):
                    g = gates0(1, j, h1_in, h0nb if first else h0_out)
                    if prev is not None:
                        phase_cell(prev[0], prev[1], c1, h1_out, h1b_out)
                    prev = (j, g)
                phase_cell(prev[0], prev[1], c1, h1_out, h1b_out)
                # logits: [d; l0; l1] = [fcW0-fcW1; fcW0; fcW1] @ h1 + bias row
                lps = lpsp.tile([3, BS], F32, tag="l")
                for kt in range(KT):
                    nc.tensor.matmul(
                        lps[:], fcw_sb[:, kt, :], h1b_out[:, kt, :],
                        start=(kt == 0), stop=False,
                    )
                nc.tensor.matmul(
                    lps[:], fcb_sb[:], ones_sb[:], start=False, stop=True
                )
                # m = 1.0 if l1 > l0 else 0.0  (d = l0 - l1 < 0);
                # written to all 4 row strips for the tiled m-matmuls
                for s in range(4):
                    nc.vector.tensor_scalar(
                        mu_sb[32 * s:32 * s + 1, :], lps[0:1, :], 0.0, None,
                        ALU.is_lt,
                    )
                nc.vector.tensor_copy(lt_sb[:], lps[:])
                tp = tpsp.tile([128, 4 * 3], F32, tag="tp")
                for bt in range(4):
                    nc.tensor.transpose(
                        tp[:, bt * 3:(bt + 1) * 3],
                        lt_sb[:, bt * 128:(bt + 1) * 128],
                        id_sb[0:3, 0:3],
                    )
                for bt in range(4):
                    nc.vector.tensor_copy(
                        lacc[:, bass.ds(bt * T * 2 + t_off * 2, 2)],
                        tp[:, bt * 3 + 1:bt * 3 + 3],
                    )

            # step 0 (x = zeros: no m-matmul); bf16 h inputs
            emit_step(0, h0ba, h0b, h1ba, h1b, h1ba, h1bb, first=True)
            if T < 8 or T % 2 != 0:
                unroll_all = True
            if unroll_all:
                for t in range(1, T):
                    if t % 2 == 1:
                        emit_step(t, h0b, h0a, h1b, h1a, h1bb, h1ba, first=False)
                    else:
                        emit_step(t, h0a, h0b, h1a, h1b, h1ba, h1bb, first=False)
            else:
                # steps 1..T-2 in ping-pong pairs
                with tc.For_i(1, T - 1, 2) as i:
                    emit_step(i, h0b, h0a, h1b, h1a, h1bb, h1ba, first=False)
                    emit_step(i + 1, h0a, h0b, h1a, h1b, h1ba, h1bb, first=False)
                # step T-1
                emit_step(T - 1, h0b, h0a, h1b, h1a, h1bb, h1ba, first=False)

            for bt in range(4):
                nc.sync.dma_start(
                    lout[bt].rearrange("p t c -> p (t c)"),
                    lacc[:, bt * T * 2:(bt + 1) * T * 2],
                )
    nc.compile()
    return nc


def pack_inputs(h, c, W_ih0, W_hh0, b_ih0, b_hh0, W_ih1, W_hh1, b_ih1, b_hh1,
                fc_W, fc_b, T):
    """Host-side packing into per-core input maps (all exact SBUF layouts)."""
    h = np.asarray(h, np.float32)
    c = np.asarray(c, np.float32)

    def gate_pack(W):
        # -> [HB, NQ, 128(p=k), KT(kt), 128(f=g)]:  W[q*1024+j*128+f, kt*128+p]
        Wr = np.asarray(W, np.float32).reshape(NQ, HB, 128, KT, 128)  # q j f kt p
        return np.ascontiguousarray(Wr.transpose(1, 0, 4, 3, 2))

    def q8(x):
        return np.clip(np.asarray(x, np.float32) * WS, -240.0, 240.0).astype(E4)

    w0f = gate_pack(W_hh0)                                   # [8,4,128,8,128]
    w0 = q8(w0f)
    w0b = w0f.astype(BF)
    w1h = gate_pack(W_hh1)
    w1i = gate_pack(W_ih1)
    w1f = np.concatenate([w1h, w1i], axis=3)                 # [8,4,128,16,128]
    w1 = q8(w1f)
    w1b = w1f.astype(BF)

    # x-feedback lhsT rows: row0 = B-A (pairs with m), row1 = A (pairs with
    # ones); pre-scaled by WS to match the fp8 gate accumulation scale.
    Wi0 = np.asarray(W_ih0, np.float32).reshape(NQ, HB, 128, C)  # q j f c
    A = Wi0[..., 0].transpose(1, 0, 2).reshape(HB, NQ, 128)
    BA = (Wi0[..., 1] - Wi0[..., 0]).transpose(1, 0, 2).reshape(HB, NQ, 128)
    xff = np.zeros((128, HB * NQ * 128), np.float32)
    for j in range(HB):
        for q in range(NQ):
            csl = slice((j * NQ + q) * 128, (j * NQ + q + 1) * 128)
            xff[32 * q, csl] = BA[j, q]
            xff[32 * q + 1, csl] = A[j, q]
    xff = q8(xff)                                            # [128, 4096]

    def bias_pack(bi, bh):
        s = (np.asarray(bi, np.float32) + np.asarray(bh, np.float32))
        return np.ascontiguousarray(
            s.reshape(NQ, HB, 128).transpose(2, 1, 0).reshape(128, HB * NQ))

    b0 = bias_pack(b_ih0, b_hh0)
    b1 = bias_pack(b_ih1, b_hh1)

    fc_W = np.asarray(fc_W, np.float32)
    fc_b = np.asarray(fc_b, np.float32)
    # columns [d, l0, l1]
    cols = np.stack([fc_W[0] - fc_W[1], fc_W[0], fc_W[1]], axis=1)  # [H, 3]
    fcw = np.ascontiguousarray(
        cols.reshape(KT, 128, 3).transpose(1, 0, 2)).astype(BF)     # [128, 8, 3]
    fcb = np.array([[fc_b[0] - fc_b[1], fc_b[0], fc_b[1]]], np.float32
                   ).astype(BF)
    ident = np.eye(128, dtype=np.float32)

    hT = h.transpose(0, 2, 1).reshape(2, KT, 128, B)   # [l, kt, p, b]
    cT = c.transpose(0, 2, 1).reshape(2, KT, 128, B)

    in_maps = []
    for i in range(NCORES):
        sl = slice(i * BS, (i + 1) * BS)
        h_sl = np.ascontiguousarray(
            hT[:, :, :, sl].transpose(0, 2, 1, 3))         # [2,128,KT,BS]
        in_maps.append({
            "hs": np.clip(h_sl, -240.0, 240.0).astype(E4),
            "hs1b": h_sl[1].astype(BF),
            "hs0b": h_sl[0].astype(BF),
            "w0pb": w0b, "w1pb": w1b,
            "cs": np.ascontiguousarray(cT[:, :, :, sl].transpose(0, 2, 1, 3)).astype(BF),
            "w0p": w0, "w1p": w1, "xf": xff, "b0": b0, "b1": b1,
            "fcw": fcw, "fcb": fcb, "ident": ident,
            "mu0": np.ones((128, BS), np.float32).astype(E4),
            "onesb": np.ones((1, BS), np.float32).astype(BF),
        })
    return in_maps


_CACHE = {}


def _run(inputs, trace=False, tmpdir=None):
    T = int(inputs["pred_len"])
    if T not in _CACHE:
        _CACHE[T] = build_kernel(T)
    nc = _CACHE[T]
    in_maps = pack_inputs(
        inputs["h"], inputs["c"], inputs["W_ih0"], inputs["W_hh0"],
        inputs["b_ih0"], inputs["b_hh0"], inputs["W_ih1"], inputs["W_hh1"],
        inputs["b_ih1"], inputs["b_hh1"], inputs["fc_W"], inputs["fc_b"], T)
    res = run_bass_kernel_spmd(
        nc, in_maps, core_ids=list(range(NCORES)), trace=trace, tmpdir=tmpdir)
    out = np.empty((B, T, C), np.float32)
    for i in range(NCORES):
        lo = res.results[i]["lout"]                    # [4, 128, T, 2]
        out[i * BS:(i + 1) * BS] = lo.reshape(BS, T, C)
    return out, res


def kernel(**inputs) -> np.ndarray:
    out, _ = _run(inputs, trace=False)
    return out


# revision 10
# speedup vs baseline: 8.9865x; 8.9865x over previous
"""Trainium2 Bass kernel for the 2-layer LSTM greedy decoder (nn_Decoder).

Strategy: data-parallel over batch (4096 -> 512 per core x 8 cores).
All recurrent state is kept feature-major in SBUF ([H partitions, batch
free]). Gate matmuls run in fp8-e4m3 with DoubleRow perf mode (2 fp8
weights per PE cell, 2x ALU rate): weights are host-scaled by 1024 into
e4m3 range and the 1/1024 descale rides the scalar-engine activation's
`scale` operand; h state is stored e4m3 (|h|<1 is native range). All LSTM
weights (12MB fp8) stay resident in SBUF - no steady-state HBM traffic.
The logits path keeps full precision: a bf16 copy of h1 feeds the fc
matmul. Argmax feedback is folded into the layer-0 gate accumulation as a
rank-2 f32r matmul update (ones/m rows, pre-scaled by 1024); biases ride
the activation's per-partition bias operand (applied after descale).
"""

import sys

sys.path.insert(0, "/opt/trn_rl_repo")

import numpy as np
import ml_dtypes

import concourse.bass as bass
import concourse.bacc as bacc
import concourse.mybir as mybir
import concourse.tile as tile
from concourse.bass_utils import run_bass_kernel_spmd

F32 = mybir.dt.float32
F32R = mybir.dt.float32r
BF16 = mybir.dt.bfloat16
FP8 = mybir.dt.float8e4
AF = mybir.ActivationFunctionType
ALU = mybir.AluOpType
DR = mybir.MatmulPerfMode.DoubleRow

H = 1024
B = 4096
C = 2
NCORES = 8
BS = B // NCORES          # 512 batch per core
KT = H // 128             # 8 k-tiles
HB = H // 128             # 8 hidden blocks
NQ = 4                    # i, f, g, o

WS = 1024.0               # host-side weight scale into e4m3 range
WSI = 1.0 / WS

BF = ml_dtypes.bfloat16
E4 = ml_dtypes.float8_e4m3


def _round_f32r(x: np.ndarray) -> np.ndarray:
    """Round fp32 to the PE's FP22 (13-bit mantissa) operand precision."""
    u = np.ascontiguousarray(x, dtype=np.float32).view(np.uint32)
    u = (u + np.uint32(0x200)) & np.uint32(0xFFFFFC00)
    return u.view(np.float32)


def build_kernel(T: int, unroll_all: bool = False):
    nc = bacc.Bacc(None, target_bir_lowering=False)

    hs = nc.dram_tensor("hs", [2, 128, KT, BS], FP8, kind="ExternalInput")
    hs1b = nc.dram_tensor("hs1b", [128, KT, BS], BF16, kind="ExternalInput")
    hs0b = nc.dram_tensor("hs0b", [128, KT, BS], BF16, kind="ExternalInput")
    w0pb = nc.dram_tensor("w0pb", [HB, NQ, 128, KT, 128], BF16, kind="ExternalInput")
    w1pb = nc.dram_tensor("w1pb", [HB, NQ, 128, 2 * KT, 128], BF16,
                          kind="ExternalInput")
    cs = nc.dram_tensor("cs", [2, 128, KT, BS], BF16, kind="ExternalInput")
    w0p = nc.dram_tensor("w0p", [HB, NQ, 128, KT, 128], FP8, kind="ExternalInput")
    w1p = nc.dram_tensor("w1p", [HB, NQ, 128, 2 * KT, 128], FP8, kind="ExternalInput")
    xf = nc.dram_tensor("xf", [128, HB * NQ * 128], FP8, kind="ExternalInput")
    b0 = nc.dram_tensor("b0", [128, HB * NQ], F32, kind="ExternalInput")
    b1 = nc.dram_tensor("b1", [128, HB * NQ], F32, kind="ExternalInput")
    fcw = nc.dram_tensor("fcw", [128, KT, 3], BF16, kind="ExternalInput")
    fcb = nc.dram_tensor("fcb", [1, 3], BF16, kind="ExternalInput")
    ident = nc.dram_tensor("ident", [128, 128], F32, kind="ExternalInput")
    onesb = nc.dram_tensor("onesb", [1, BS], BF16, kind="ExternalInput")
    mu0 = nc.dram_tensor("mu0", [128, BS], FP8, kind="ExternalInput")
    lout = nc.dram_tensor("lout", [4, 128, T, 2], F32, kind="ExternalOutput")

    with tile.TileContext(nc) as tc:
        with (
            tc.tile_pool(name="st", bufs=1) as st,
            tc.tile_pool(name="wst", bufs=2) as wst,
            tc.tile_pool(name="tmp", bufs=2) as tmp,
            tc.tile_pool(name="gps", bufs=6, space="PSUM") as gps,
            tc.tile_pool(name="lps", bufs=1, space="PSUM") as lpsp,
            tc.tile_pool(name="tps", bufs=1, space="PSUM") as tpsp,
        ):
            # Persistent state (ping-pong h buffers; c updated in place)
            h0a = st.tile([128, KT, BS], FP8, tag="h0a")
            h0b = st.tile([128, KT, BS], FP8, tag="h0b")
            h1a = st.tile([128, KT, BS], FP8, tag="h1a")
            h1b = st.tile([128, KT, BS], FP8, tag="h1b")
            h0ba = st.tile([128, KT, BS], BF16, tag="h0ba")  # bf16 h0 init
            h0nb = st.tile([128, KT, BS], BF16, tag="h0nb")  # bf16 h0n (step 0)
            h1ba = st.tile([128, KT, BS], BF16, tag="h1ba")  # bf16 h1 (logits)
            h1bb = st.tile([128, KT, BS], BF16, tag="h1bb")
            c0 = st.tile([128, KT, BS], BF16, tag="c0")
            c1 = st.tile([128, KT, BS], BF16, tag="c1")
            w0_sb = st.tile([128, HB, NQ, KT, 128], FP8, tag="w0")
            w1_sb = st.tile([128, HB, NQ, 2 * KT, 128], FP8, tag="w1")
            xf_sb = st.tile([128, HB * NQ * 128], FP8, tag="xf")
            b0_sb = st.tile([128, HB * NQ], F32, tag="b0")
            b1_sb = st.tile([128, HB * NQ], F32, tag="b1")
            fcw_sb = st.tile([128, KT, 3], BF16, tag="fcw")
            fcb_sb = st.tile([1, 3], BF16, tag="fcb")
            id_sb = st.tile([128, 128], F32, tag="ident")
            mu_sb = st.tile([128, BS], FP8, tag="mu")  # rows 32q=m, 32q+1=ones
            ones_sb = st.tile([1, BS], BF16, tag="ones")
            lt_sb = st.tile([3, BS], F32, tag="lt")    # rows d, l0, l1
            lacc = st.tile([128, 4 * T * 2], F32, tag="lacc")

            nc.sync.dma_start(h0a[:], hs[0])
            nc.sync.dma_start(h1a[:], hs[1])
            nc.sync.dma_start(h1ba[:], hs1b[:])
            nc.sync.dma_start(h0ba[:], hs0b[:])
            nc.sync.dma_start(c0[:], cs[0])
            nc.sync.dma_start(c1[:], cs[1])
            for j in range(HB):
                for q in range(NQ):
                    nc.sync.dma_start(w0_sb[:, j, q], w0p[j, q])
                    nc.sync.dma_start(w1_sb[:, j, q], w1p[j, q])
            nc.sync.dma_start(xf_sb[:], xf[:])
            nc.sync.dma_start(b0_sb[:], b0[:])
            nc.sync.dma_start(b1_sb[:], b1[:])
            nc.sync.dma_start(fcw_sb[:], fcw[:])
            nc.sync.dma_start(fcb_sb[:], fcb[:])
            nc.sync.dma_start(id_sb[:], ident[:])
            # row1 stays 1.0 forever; row0 (m) is overwritten by is_lt each
            # step before any matmul reads it (step 0 skips the m-matmul).
            nc.sync.dma_start(mu_sb[:], mu0[:])
            nc.sync.dma_start(ones_sb[:], onesb[:])

            def phase_gates(layer, j, h_in, h_aux, with_m):
                """Gate matmuls + activations for hidden block j of one layer.

                layer 0: contraction = W_hh0 @ h_in (+ x feedback via m-matmul)
                layer 1: contraction = W_hh1 @ h_in then W_ih1 @ h_aux
                All gate matmuls are fp8 DoubleRow over k-tile pairs.
                """
                nkt = KT if layer == 0 else 2 * KT
                wsrc = w0_sb if layer == 0 else w1_sb
                bias = b0_sb if layer == 0 else b1_sb
                gact = tmp.tile([128, NQ, BS], BF16, tag="gact")
                for q in range(NQ):
                    w = wsrc[:, j, q]
                    ps = gps.tile([128, BS], F32, tag="g")
                    for m in range(nkt // 2):
                        rhs = h_in if m < KT // 2 else h_aux
                        kk = (2 * m) % KT
                        last = (m == nkt // 2 - 1) and not (layer == 0 and with_m)
                        nc.tensor.matmul(
                            ps[:],
                            w[:, 2 * m:2 * m + 2, :],
                            rhs[:, kk:kk + 2, :],
                            start=(m == 0),
                            stop=last,
                            perf_mode=DR,
                        )
                    if layer == 0 and with_m:
                        idx = j * NQ + q
                        nc.tensor.matmul(
                            ps[:],
                            xf_sb[32 * q:32 * q + 2, idx * 128:(idx + 1) * 128],
                            mu_sb[32 * q:32 * q + 2, :],
                            start=False,
                            stop=True,
                            tile_position=(32 * q, 0),
                        )
                    fn = AF.Tanh if q == 2 else AF.Sigmoid
                    idx = j * NQ + q
                    nc.scalar.activation(
                        gact[:, q, :], ps[:], fn,
                        bias=bias[:, idx:idx + 1], scale=WSI,
                    )
                return gact

            def phase_gates_precise(layer, j, h_in, h_aux):
                """Step-0 gate matmuls in bf16 (initial h is N(0,1)-scale;
                fp8 would inject a large decaying transient)."""
                nkt = KT if layer == 0 else 2 * KT
                wsrc = w0pb if layer == 0 else w1pb
                bias = b0_sb if layer == 0 else b1_sb
                gact = tmp.tile([128, NQ, BS], BF16, tag="gact")
                for q in range(NQ):
                    w = wst.tile([128, 2 * KT, 128], BF16, tag="w")
                    nc.sync.dma_start(w[:, :nkt, :], wsrc[j, q])
                    ps = gps.tile([128, BS], F32, tag="g")
                    for kt in range(nkt):
                        rhs = h_in if kt < KT else h_aux
                        nc.tensor.matmul(
                            ps[:],
                            w[:, kt, :],
                            rhs[:, kt % KT, :],
                            start=(kt == 0),
                            stop=(kt == nkt - 1),
                        )
                    fn = AF.Tanh if q == 2 else AF.Sigmoid
                    idx = j * NQ + q
                    nc.scalar.activation(
                        gact[:, q, :], ps[:], fn, bias=bias[:, idx:idx + 1]
                    )
                return gact

            def phase_cell(j, gact, c_st, h_out, h_out_b=None):
                t1 = tmp.tile([128, BS], BF16, tag="t1")
                t2 = tmp.tile([128, BS], BF16, tag="t2")
                ct = tmp.tile([128, BS], BF16, tag="ct")
                nc.vector.tensor_mul(t1[:], gact[:, 1, :], c_st[:, j, :])
                nc.vector.tensor_mul(t2[:], gact[:, 0, :], gact[:, 2, :])
                nc.vector.tensor_add(c_st[:, j, :], t1[:], t2[:])
                nc.scalar.activation(ct[:], c_st[:, j, :], AF.Tanh)
                nc.vector.tensor_mul(h_out[:, j, :], gact[:, 3, :], ct[:])
                if h_out_b is not None:
                    nc.vector.tensor_mul(h_out_b[:, j, :], gact[:, 3, :], ct[:])

            def emit_step(t_off, h0_in, h0_out, h1_in, h1_out, h1b_in, h1b_out,
                          first):
                gates0 = (phase_gates_precise if first else
                          lambda l, j, hi, ha: phase_gates(l, j, hi, ha, False))
                # one-block skew: block j's cell tail is emitted after block
                # j+1's gate phase, keeping tanh(c)'s DVE-wait off the ACT
                # engine's head-of-line and freeing PSUM banks promptly.
                h0_out_b = h0nb if first else None
                prev = None
                for j in range(HB):
                    if first:
                        g = gates0(0, j, h0_in, None)
                    else:
                        g = phase_gates(0, j, h0_in, None, True)
                    if prev is not None:
                        phase_cell(prev[0], prev[1], c0, h0_out, h0_out_b)
                    prev = (j, g)
                phase_cell(prev[0], prev[1], c0, h0_out, h0_out_b)
                prev = None
                for j in range(HB):
                    g = gates0(1, j, h1_in, h0nb if first else h0_out)
                    if prev is not None:
                        phase_cell(prev[0], prev[1], c1, h1_out, h1b_out)
                    prev = (j, g)
                phase_cell(prev[0], prev[1], c1, h1_out, h1b_out)
                # logits: [d; l0; l1] = [fcW0-fcW1; fcW0; fcW1] @ h1 + bias row
                lps = lpsp.tile([3, BS], F32, tag="l")
                for kt in range(KT):
                    nc.tensor.matmul(
                        lps[:], fcw_sb[:, kt, :], h1b_out[:, kt, :],
                        start=(kt == 0), stop=False,
                    )
                nc.tensor.matmul(
                    lps[:], fcb_sb[:], ones_sb[:], start=False, stop=True
                )
                # m = 1.0 if l1 > l0 else 0.0  (d = l0 - l1 < 0);
                # written to all 4 row strips for the tiled m-matmuls
                for s in range(4):
                    nc.vector.tensor_scalar(
                        mu_sb[32 * s:32 * s + 1, :], lps[0:1, :], 0.0, None,
                        ALU.is_lt,
                    )
                nc.vector.tensor_copy(lt_sb[:], lps[:])
                tp = tpsp.tile([128, 4 * 3], F32, tag="tp")
                for bt in range(4):
                    nc.tensor.transpose(
                        tp[:, bt * 3:(bt + 1) * 3],
                        lt_sb[:, bt * 128:(bt + 1) * 128],
                        id_sb[0:3, 0:3],
                    )
                for bt in range(4):
                    nc.vector.tensor_copy(
                        lacc[:, bass.ds(bt * T * 2 + t_off * 2, 2)],
                        tp[:, bt * 3 + 1:bt * 3 + 3],
                    )

            # step 0 (x = zeros: no m-matmul); bf16 h inputs
            emit_step(0, h0ba, h0b, h1ba, h1b, h1ba, h1bb, first=True)
            if T < 8 or T % 2 != 0:
                unroll_all = True
            if unroll_all:
                for t in range(1, T):
                    if t % 2 == 1:
                        emit_step(t, h0b, h0a, h1b, h1a, h1bb, h1ba, first=False)
                    else:
                        emit_step(t, h0a, h0b, h1a, h1b, h1ba, h1bb, first=False)
            else:
                # steps 1..T-2 in ping-pong pairs
                with tc.For_i(1, T - 1, 2) as i:
                    emit_step(i, h0b, h0a, h1b, h1a, h1bb, h1ba, first=False)
                    emit_step(i + 1, h0a, h0b, h1a, h1b, h1ba, h1bb, first=False)
                # step T-1
                emit_step(T - 1, h0b, h0a, h1b, h1a, h1bb, h1ba, first=False)

            for bt in range(4):
                nc.sync.dma_start(
                    lout[bt].rearrange("p t c -> p (t c)"),
                    lacc[:, bt * T * 2:(bt + 1) * T * 2],
                )
    nc.compile()
    return nc


def pack_inputs(h, c, W_ih0, W_hh0, b_ih0, b_hh0, W_ih1, W_hh1, b_ih1, b_hh1,
                fc_W, fc_b, T):
    """Host-side packing into per-core input maps (all exact SBUF layouts)."""
    h = np.asarray(h, np.float32)
    c = np.asarray(c, np.float32)

    def gate_pack(W):
        # -> [HB, NQ, 128(p=k), KT(kt), 128(f=g)]:  W[q*1024+j*128+f, kt*128+p]
        Wr = np.asarray(W, np.float32).reshape(NQ, HB, 128, KT, 128)  # q j f kt p
        return np.ascontiguousarray(Wr.transpose(1, 0, 4, 3, 2))

    def q8(x):
        return np.clip(np.asarray(x, np.float32) * WS, -240.0, 240.0).astype(E4)

    w0f = gate_pack(W_hh0)                                   # [8,4,128,8,128]
    w0 = q8(w0f)
    w0b = w0f.astype(BF)
    w1h = gate_pack(W_hh1)
    w1i = gate_pack(W_ih1)
    w1f = np.concatenate([w1h, w1i], axis=3)                 # [8,4,128,16,128]
    w1 = q8(w1f)
    w1b = w1f.astype(BF)

    # x-feedback lhsT rows: row0 = B-A (pairs with m), row1 = A (pairs with
    # ones); pre-scaled by WS to match the fp8 gate accumulation scale.
    Wi0 = np.asarray(W_ih0, np.float32).reshape(NQ, HB, 128, C)  # q j f c
    A = Wi0[..., 0].transpose(1, 0, 2).reshape(HB, NQ, 128)
    BA = (Wi0[..., 1] - Wi0[..., 0]).transpose(1, 0, 2).reshape(HB, NQ, 128)
    xff = np.zeros((128, HB * NQ * 128), np.float32)
    for j in range(HB):
        for q in range(NQ):
            csl = slice((j * NQ + q) * 128, (j * NQ + q + 1) * 128)
            xff[32 * q, csl] = BA[j, q]
            xff[32 * q + 1, csl] = A[j, q]
    xff = q8(xff)                                            # [128, 4096]

    def bias_pack(bi, bh):
        s = (np.asarray(bi, np.float32) + np.asarray(bh, np.float32))
        return np.ascontiguousarray(
            s.reshape(NQ, HB, 128).transpose(2, 1, 0).reshape(128, HB * NQ))

    b0 = bias_pack(b_ih0, b_hh0)
    b1 = bias_pack(b_ih1, b_hh1)

    fc_W = np.asarray(fc_W, np.float32)
    fc_b = np.asarray(fc_b, np.float32)
    # columns [d, l0, l1]
    cols = np.stack([fc_W[0] - fc_W[1], fc_W[0], fc_W[1]], axis=1)  # [H, 3]
    fcw = np.ascontiguousarray(
        cols.reshape(KT, 128, 3).transpose(1, 0, 2)).astype(BF)     # [128, 8, 3]
    fcb = np.array([[fc_b[0] - fc_b[1], fc_b[0], fc_b[1]]], np.float32
                   ).astype(BF)
    ident = np.eye(128, dtype=np.float32)

    hT = h.transpose(0, 2, 1).reshape(2, KT, 128, B)   # [l, kt, p, b]
    cT = c.transpose(0, 2, 1).reshape(2, KT, 128, B)

    in_maps = []
    for i in range(NCORES):
        sl = slice(i * BS, (i + 1) * BS)
        h_sl = np.ascontiguousarray(
            hT[:, :, :, sl].transpose(0, 2, 1, 3))         # [2,128,KT,BS]
        in_maps.append({
            "hs": np.clip(h_sl, -240.0, 240.0).astype(E4),
            "hs1b": h_sl[1].astype(BF),
            "hs0b": h_sl[0].astype(BF),
            "w0pb": w0b, "w1pb": w1b,
            "cs": np.ascontiguousarray(cT[:, :, :, sl].transpose(0, 2, 1, 3)).astype(BF),
            "w0p": w0, "w1p": w1, "xf": xff, "b0": b0, "b1": b1,
            "fcw": fcw, "fcb": fcb, "ident": ident,
            "mu0": np.ones((128, BS), np.float32).astype(E4),
            "onesb": np.ones((1, BS), np.float32).astype(BF),
        })
    return in_maps


_CACHE = {}


def _run(inputs, trace=False, tmpdir=None):
    T = int(inputs["pred_len"])
    if T not in _CACHE:
        _CACHE[T] = build_kernel(T)
    nc = _CACHE[T]
    in_maps = pack_inputs(
        inputs["h"], inputs["c"], inputs["W_ih0"], inputs["W_hh0"],
        inputs["b_ih0"], inputs["b_hh0"], inputs["W_ih1"], inputs["W_hh1"],
        inputs["b_ih1"], inputs["b_hh1"], inputs["fc_W"], inputs["fc_b"], T)
    res = run_bass_kernel_spmd(
        nc, in_maps, core_ids=list(range(NCORES)), trace=trace, tmpdir=tmpdir)
    out = np.empty((B, T, C), np.float32)
    for i in range(NCORES):
        lo = res.results[i]["lout"]                    # [4, 128, T, 2]
        out[i * BS:(i + 1) * BS] = lo.reshape(BS, T, C)
    return out, res


def kernel(**inputs) -> np.ndarray:
    out, _ = _run(inputs, trace=False)
    return out
